# revision 3
# baseline (speedup 1.0000x reference)
"""Trainium2 Bass kernel v2: end-to-end model (pool -> linear -> max/argmax ->
top-k -> gather), query-sharded across 8 NeuronCores.

Sharding: x batch-sharded (8 samples/core) for pooling; pooled features
AllGathered (307KB); W output-column (query) sharded 38-of-304/core so each
core streams only 15.3MB of W (vs 121MB replicated); per-query results
AllToAll'd (68KB) so each core runs topk+gather for its own 8 samples.

Self-contained: hardcodes all shapes; builds one SPMD Bass program and runs
it via run_bass_kernel_spmd on cores 0-7.
"""

import os
import sys
from contextlib import ExitStack

import numpy as np

for _p in ("/opt/trn_rl_repo", "/root/.axon_site/_ro/trn_rl_repo"):
    if os.path.isdir(_p) and _p not in sys.path:
        sys.path.append(_p)

import concourse.bass as bass
import concourse.tile as tile
from concourse import bacc, library_config, mybir
from concourse.bass_utils import run_bass_kernel_spmd

dt = mybir.dt
F32 = dt.float32
AX = mybir.AxisListType
OP = mybir.AluOpType

# ---------------- problem constants (hardcoded) ----------------
B, CHN, HIMG, WIMG = 64, 3, 640, 640
NQ, NCHAN, NCL, TOPK = 300, 84, 80, 150
KDIM = 1200
NCORES = 8
BPC = B // NCORES                  # samples per core = 8
KT, KTS = 10, 120                  # K tiling: 10 x 120
NQP = 304                          # padded query count (8*38)
NQC = NQP // NCORES                # queries per core = 38
NCOLS = NQC * NCHAN                # 3192 matmul columns per core
CHUNKQ = [6, 6, 6, 6, 6, 6, 2]     # queries per GEMM chunk (6*84=504 <= 512 psum)
SCALE = np.float64(1.0) / (32 * 32 * 255)
NEG = -3.0e38
NIDX = 160                         # padded top-k index count (152 used)
NROUND = 19                        # 19 rounds x 8 = 152 >= 150
FEAT = NQC * 6                     # 228 feat cols per (dest, sample)
BLK = FEAT + NQC                   # 266 f32 per all-to-all block row

# GEMM mode: "bf3" = 3-pass bf16 decomposition (y = Wh@ph + Wl@ph + Wh@pl,
# exact to ~2^-16 since bf16 products accumulate in f32 psum; 3 PE cycles/col
# vs f32's 4), "f32" = plain f32 matmul (slower, max precision)
MM_MODE = os.environ.get("KERNEL_MM", "bf3")
BF16 = dt.bfloat16


def build_program():
    nc = bacc.Bacc(
        "TRN2", target_bir_lowering=False, debug=False, num_devices=NCORES
    )
    # x host-packed to uint8 partition-major: 4x fewer HBM bytes, linear reads
    x_d = nc.dram_tensor(
        "x", [BPC, CHN, 128, 5, WIMG], dt.uint8, kind="ExternalInput"
    )
    # per-core query slice of W, K-permuted on host, tile-contiguous
    if MM_MODE == "bf3":
        w_d = nc.dram_tensor("w", [KT, KTS, 2, NCOLS], BF16, kind="ExternalInput")
    else:
        w_d = nc.dram_tensor("w", [KT, KTS, NCOLS], F32, kind="ExternalInput")
    g4_d = nc.dram_tensor("g4", [128, 4], F32, kind="ExternalInput")
    id64_d = nc.dram_tensor("id64", [64, 64], F32, kind="ExternalInput")
    iod_d = nc.dram_tensor("iod", [64, NCL], F32, kind="ExternalInput")
    qmask_d = nc.dram_tensor("qmask", [64, NQC], F32, kind="ExternalInput")
    out_d = nc.dram_tensor("out", [BPC, TOPK, 6], F32, kind="ExternalOutput")

    with tile.TileContext(nc) as tc:
        with ExitStack() as ctx:
            _body(ctx, tc, x_d, w_d, g4_d, id64_d, iod_d, qmask_d, out_d)
    nc.finalize()
    return nc


def _body(ctx, tc, x_d, w_d, g4_d, id64_d, iod_d, qmask_d, out_d):
    nc = tc.nc

    # ---------------- persistent tiles ----------------
    P = ctx.enter_context(tc.tile_pool(name="persist", bufs=1))
    DCC = ctx.enter_context(tc.tile_pool(name="dcc", bufs=1, space="DRAM"))

    g4 = P.tile([128, 4], F32, tag="g4")
    id64 = P.tile([64, 64], F32, tag="id64")
    iod = P.tile([64, NCL], F32, tag="iod")
    qmask = P.tile([64, NQC], F32, tag="qmask")

    pall_a = P.tile([32, KDIM], F32, tag="pall_a")       # pooled, images 0-3 x 8 cores
    pall_b = P.tile([32, KDIM], F32, tag="pall_b")       # pooled, images 4-7 x 8 cores
    if MM_MODE == "bf3":
        ph = [P.tile([KTS, 64], BF16, tag=f"ph{k}", name=f"ph{k}") for k in range(KT)]
        pl = [P.tile([KTS, 64], BF16, tag=f"pl{k}", name=f"pl{k}") for k in range(KT)]
        wt = [P.tile([KTS, 2 * NCOLS], BF16, tag=f"wt{k}", name=f"wt{k}") for k in range(KT)]
    else:
        pt = [P.tile([KTS, 64], F32, tag=f"pt{k}", name=f"pt{k}") for k in range(KT)]
        wt = [P.tile([KTS, NCOLS], F32, tag=f"wt{k}", name=f"wt{k}") for k in range(KT)]
    feat6 = P.tile([64, FEAT], F32, tag="feat6")         # [sample, (q, 6)] own queries
    scores6 = P.tile([64, NQC], F32, tag="scores6")      # per-query max score
    eq = P.tile([64, 6 * NCL], F32, tag="eq")            # argmax scratch
    am = P.tile([64, 6 * NCL], F32, tag="am")
    argt = P.tile([64, 6], F32, tag="argt")
    feat = P.tile([128, NQP * 6], F32, tag="feat")       # gather source [p, q, 6]
    swk = P.tile([BPC, NQP], F32, tag="swk")             # topk scratch (destroyed)
    tv = P.tile([BPC, NROUND * 8], F32, tag="tv")        # topk values (desc)
    ti = P.tile([BPC, NROUND * 8], dt.uint32, tag="ti")  # topk indices
    ti16 = P.tile([BPC, NIDX], dt.int16, tag="ti16")
    wrap = P.tile([128, NIDX // 16], dt.int16, tag="wrap")
    gout = P.tile([128, NIDX * 6], F32, tag="gout")

    # collective bounce buffers (DRAM)
    ag_in = DCC.tile([BPC, KDIM], F32, tag="ag_in")
    ag1_out = DCC.tile([32, KDIM], F32, tag="ag1_out")
    ag2_out = DCC.tile([32, KDIM], F32, tag="ag2_out")
    aa_in = DCC.tile([64, BLK], F32, tag="aa_in")
    aa_out = DCC.tile([64, BLK], F32, tag="aa_out")

    nc.vector.memset(ti16[:, :], 0)
    nc.vector.memset(feat[:, :], 0)  # only partitions 16b hold real data
    # load the gather library up front so no drain+reload lands in the tail
    nc.gpsimd.load_library(library_config.ap_gather)

    def _wdma(k, eng):
        if MM_MODE == "bf3":
            eng.dma_start(
                wt[k][:].rearrange("p (h n) -> p h n", h=2), w_d[k]
            )
        else:
            eng.dma_start(wt[k][:], w_d[k])

    # ---------------- phase 1: pooling (x -> ag_in [8,1200]) ----------------
    # K layout of a pooled row: k = i*300 + cx*100 + t*20 + j  (i=p//32,
    # cx=BGR channel, t=row tile, j=width group). The host W permutation
    # absorbs this AND the BGR->RGB flip, so the device never reorders.
    with tc.tile_pool(name="xp", bufs=12) as XP, \
         tc.tile_pool(name="s1p", bufs=6) as S1P, \
         tc.tile_pool(name="smallp", bufs=4) as SMALL, \
         tc.tile_pool(name="pps", bufs=4, space="PSUM") as PPS:
        # issue ALL x DMAs up front, alternating both HWDGE queues so the
        # first tiles land 2x sooner; nothing here may wait on pooling
        # compute, or the x stream serializes behind it
        xas = []
        for b in range(BPC):
            for cx in range(CHN):
                xa = XP.tile([128, 5 * WIMG], dt.uint8, tag="xa", name="xa")
                eng = nc.sync if (b * CHN + cx) % 2 == 0 else nc.scalar
                eng.dma_start(
                    xa[:].rearrange("p (t w) -> p t w", t=5), x_d[b, cx]
                )
                xas.append(xa)
        # constants next (needed from ~25us on; they land right behind x)
        nc.sync.dma_start(g4[:], g4_d[:])
        nc.sync.dma_start(id64[:], id64_d[:])
        nc.sync.dma_start(iod[:], iod_d[:])
        nc.sync.dma_start(qmask[:], qmask_d[:])
        # the whole W stream rides the sync queue behind x: the scalar
        # queue stays free for pooling pc/scatters (a W trigger's DGE
        # stall would delay them and so the first AllGather), and W isn't
        # consumed until after the second AllGather anyway
        for k in range(KT):
            _wdma(k, nc.sync)

        for b in range(BPC):
            ps = PPS.tile([4, 3 * 100], F32, tag="ps", name="ps")
            pc = SMALL.tile([4, 3 * 100], F32, tag="pc", name="pc")
            for cx in range(CHN):
                xa = xas[b * CHN + cx]
                s1f = S1P.tile([128, 100], F32, tag="s1f", name="s1f")
                # width pooling: sum groups of 32 -> [128, (t,j)] f32 (exact:
                # sums of 32 uint8 <= 8160 < 2^24)
                with nc.allow_low_precision(reason="f32 sums of uint8 are exact"):
                    nc.vector.tensor_reduce(
                        s1f[:],
                        xa[:].rearrange("p (t j g) -> p t j g", t=5, j=20),
                        axis=AX.X,
                        op=OP.add,
                    )
                # height pooling via PE: G4.T @ s1f -> [4, 100] (scaled)
                nc.tensor.matmul(
                    ps[:, cx * 100 : (cx + 1) * 100], g4[:], s1f[:],
                    start=True, stop=True,
                )
            nc.scalar.copy(pc[:], ps[:])
            # one scatter DMA per image, straight into the AllGather input:
            # ag_in[b] row-major = (i, cx, t, j) = the k layout above
            nc.scalar.dma_start(
                ag_in[b].rearrange("(i r) -> i r", i=4), pc[:]
            )

    # ---------------- phase 2: AllGather pooled, split in 2 ----------------
    # AG#1 covers images 0-3 and fires while images 4-7 are still pooling,
    # hiding roughly half the collective latency + cross-core skew.
    # Sample order becomes interleaved: pall_a row 4d+u = sample 8d+u,
    # pall_b row 4d+u = sample 8d+4+u; the lhsT column (= psum partition)
    # carries that order, undone by the AllToAll staging DMAs below.
    nc.gpsimd.collective_compute(
        "AllGather",
        OP.bypass,
        replica_groups=[list(range(NCORES))],
        ins=[ag_in[0:4, :]],
        outs=[ag1_out[:]],
    )
    nc.gpsimd.collective_compute(
        "AllGather",
        OP.bypass,
        replica_groups=[list(range(NCORES))],
        ins=[ag_in[4:8, :]],
        outs=[ag2_out[:]],
    )
    nc.sync.dma_start(pall_a[:], ag1_out[:])
    nc.sync.dma_start(pall_b[:], ag2_out[:])

    # transpose pall halves -> lhsT tiles [120, 64] for the main matmul
    with tc.tile_pool(name="pts", bufs=2, space="PSUM") as PTS:
        for k in range(KT):
            pst = PTS.tile([KTS, 64], F32, tag="pst", name="pst")
            nc.tensor.transpose(
                pst[:, 0:32], pall_a[:, k * KTS : (k + 1) * KTS], id64[0:32, 0:32]
            )
            nc.tensor.transpose(
                pst[:, 32:64], pall_b[:, k * KTS : (k + 1) * KTS], id64[0:32, 0:32]
            )
            # the psum->sbuf copy also un-interleaves the AG-split sample
            # order: lhsT col 8d+4*half+u <- pst col 32*half+4d+u, so the
            # GEMM's psum partitions are in true sample order downstream
            def _unperm(ap):  # [120, 64] -> [120, (half, d, u)] iteration view
                return ap.rearrange("p (d two u) -> p two d u", two=2, u=4)

            def _src(ap):
                return ap.rearrange("p (two d u) -> p two d u", d=8, u=4)

            if MM_MODE == "bf3":
                nc.scalar.copy(_unperm(ph[k][:]), _src(pst[:]))  # hi = bf16(p)
                nc.vector.tensor_tensor(                         # lo = bf16(p - hi)
                    _unperm(pl[k][:]), _src(pst[:]), _unperm(ph[k][:]),
                    op=OP.subtract,
                )
            else:
                nc.scalar.copy(_unperm(pt[k][:]), _src(pst[:]))

    # ------------- phase 3: main matmul + per-chunk postproc -----------
    fv = feat6[:].rearrange("p (q c) -> p q c", c=6)
    q0s = np.cumsum([0] + CHUNKQ).tolist()
    with tc.tile_pool(name="yps", bufs=3, space="PSUM") as YPS:
        # chunk-major: W is fully resident well before the AllGather lands,
        # so no arrival pacing is needed; chunk j's postproc (DVE) then
        # overlaps the PE running chunk j+1
        for j, nq in enumerate(CHUNKQ):
            psy = YPS.tile([64, 6 * NCHAN], F32, tag="psy", name="psy")
            c0 = q0s[j] * NCHAN
            for k in range(KT):
                if MM_MODE == "bf3":
                    for pi, (lhs, off) in enumerate(
                        [(ph[k], 0), (pl[k], 0), (ph[k], NCOLS)]
                    ):
                        nc.tensor.matmul(
                            psy[:, : nq * NCHAN],
                            lhs[:],
                            wt[k][:, off + c0 : off + c0 + nq * NCHAN],
                            start=(k == 0 and pi == 0),
                            stop=(k == KT - 1 and pi == 2),
                        )
                else:
                    nc.tensor.matmul(
                        psy[:, : nq * NCHAN],
                        pt[k][:],
                        wt[k][:, c0 : c0 + nq * NCHAN],
                        start=(k == 0),
                        stop=(k == KT - 1),
                    )
            q0 = q0s[j]
            psv = psy[:, : nq * NCHAN].rearrange("b (q c) -> b q c", q=nq)
            # boxes -> feat6[:, q, 0:4]
            nc.vector.tensor_copy(fv[:, q0 : q0 + nq, 0:4], psv[:, :, 0:4])
            # per-query max score -> scores6
            nc.vector.tensor_reduce(
                scores6[:, q0 : q0 + nq], psv[:, :, 4:NCHAN],
                axis=AX.X, op=OP.max,
            )
            # argmax id: first class attaining the max (descending-iod trick)
            eqv = eq[:, : nq * NCL].rearrange("p (q c) -> p q c", q=nq)
            nc.vector.tensor_tensor(
                eqv, psv[:, :, 4:NCHAN],
                scores6[:, q0 : q0 + nq].unsqueeze(-1).broadcast_to((64, nq, NCL)),
                op=OP.is_ge,
            )
            amv = am[:, : nq * NCL].rearrange("p (q c) -> p q c", q=nq)
            nc.vector.tensor_tensor(
                amv, eqv,
                iod[:].unsqueeze(1).broadcast_to((64, nq, NCL)),
                op=OP.mult,
            )
            nc.vector.tensor_reduce(argt[:, :nq], amv, axis=AX.X, op=OP.max)
            # id = 79 - arg -> feat6[:, q, 5]
            nc.vector.tensor_scalar(
                fv[:, q0 : q0 + nq, 5], argt[:, :nq], -1.0, float(NCL - 1),
                op0=OP.mult, op1=OP.add,
            )

    # mask pad queries (core 7 only; qmask is 0 elsewhere; adds are exact)
    nc.vector.tensor_tensor(scores6[:], scores6[:], qmask[:], op=OP.add)
    nc.vector.tensor_copy(fv[:, :, 4], scores6[:])

    # ------------- phase 4: AllToAll per-query results -----------------
    # block layout per (dest core, sample): [0:228]=feat6 row, [228:266]=scores
    nc.sync.dma_start(aa_in[:, 0:FEAT], feat6[:])
    nc.scalar.dma_start(aa_in[:, FEAT:BLK], scores6[:])
    nc.gpsimd.collective_compute(
        "AllToAll",
        OP.bypass,
        replica_groups=[list(range(NCORES))],
        ins=[aa_in[:]],
        outs=[aa_out[:]],
    )
    # aa_out = [src core, my sample, 266]; reassemble:
    av = aa_out[:].rearrange("(s b) e -> b s e", s=NCORES)
    # topk working scores first — they gate the serial topk chain
    nc.sync.dma_start(
        swk[:].rearrange("b (s q) -> b s q", s=NCORES),
        av[:, :, FEAT:BLK],
    )
    # feat rows: partition 16b gets [304 queries, 6] contiguous
    nc.scalar.dma_start(
        feat[:].rearrange("(b o) f -> b o f", o=16)[:, 0, :]
               .rearrange("b (s e) -> b s e", s=NCORES),
        av[:, :, 0:FEAT],
    )

    # ---------------- phase 5: top-150 via iterated max8 ----------------
    # index wrap (ti16 -> DRAM -> wrap) is pipelined in 2 halves so most of
    # it hides under the later topk rounds
    with tc.tile_pool(name="dscr", bufs=1, space="DRAM") as DSCR:
        # on-chip transpose to wrap layout (one strided DVE copy), then the
        # DRAM bounce DMAs are plain 2-dim copies
        tsc = DSCR.tile([BPC, NIDX], dt.int16, tag="tsc")
        tip = P.tile([BPC, NIDX], dt.int16, tag="tip")
        for r in range(NROUND):
            nc.vector.max(tv[:, 8 * r : 8 * r + 8], swk[:, :])
            nc.vector.max_index(ti[:, 8 * r : 8 * r + 8], tv[:, 8 * r : 8 * r + 8], swk[:, :])
            if r < NROUND - 1:
                nc.vector.match_replace(
                    swk[:, :], tv[:, 8 * r : 8 * r + 8], swk[:, :], NEG
                )
        nc.vector.tensor_copy(ti16[:, : NROUND * 8], ti[:, :])
        # tip[b, p*10 + f] = ti16[b, f*16 + p]  (wrap layout, p-major)
        nc.vector.tensor_copy(
            tip[:].rearrange("b (p f) -> b p f", f=NIDX // 16),
            ti16[:].rearrange("b (f p) -> b p f", p=16),
        )
        nc.sync.dma_start(tsc[:], tip[:])
        nc.sync.dma_start(
            wrap[:], tsc[:].rearrange("b (p f) -> (b p) f", f=NIDX // 16)
        )

    # ---------------- phase 6: gather + output ----------------
    nc.gpsimd.ap_gather(
        gout[:].rearrange("p (i c) -> p i c", c=6),
        feat[:].rearrange("p (q c) -> p q c", c=6),
        wrap[:],
        channels=128,
        num_elems=NQP,
        d=6,
        num_idxs=NIDX,
    )
    # one DMA: partition 16b row -> out sample b
    nc.sync.dma_start(
        out_d[:].rearrange("b k c -> b (k c)"),
        gout[:].rearrange("(b o) f -> b o f", o=16)[:, 0, : TOPK * 6],
    )


def _k_perm():
    """k_new = i*300 + cx*100 + t*20 + j -> k_orig = (2-cx)*400 + (t*4+i)*20 + j."""
    perm = np.empty(KDIM, np.int64)
    for i in range(4):
        for cx in range(CHN):
            for t in range(5):
                for j in range(20):
                    k_new = i * 300 + cx * 100 + t * 20 + j
                    perm[k_new] = (2 - cx) * 400 + (t * 4 + i) * 20 + j
    return perm


def _make_consts():
    g4 = np.zeros((128, 4), np.float32)
    for i in range(4):
        g4[32 * i : 32 * (i + 1), i] = np.float32(SCALE)
    id64 = np.eye(64, dtype=np.float32)
    iod = np.broadcast_to(
        (np.float32(NCL - 1) - np.arange(NCL, dtype=np.float32))[None, :], (64, NCL)
    ).copy()
    return g4, id64, iod


_NC_CACHE = {}


def _get_nc():
    key = MM_MODE
    if key not in _NC_CACHE:
        _NC_CACHE[key] = build_program()
    return _NC_CACHE[key]


def pack_w(W: np.ndarray) -> list[np.ndarray]:
    """[1200, 25200] -> per-core [KT, 120, (2,) 3192], K-permuted, query-padded."""
    import ml_dtypes

    Wp = W[_k_perm()].reshape(KDIM, NQ, NCHAN)
    Wpad = np.zeros((KDIM, NQP, NCHAN), np.float32)
    Wpad[:, :NQ, :] = Wp
    out = []
    for c in range(NCORES):
        ws = Wpad[:, c * NQC : (c + 1) * NQC, :].reshape(KT, KTS, NCOLS)
        if MM_MODE == "bf3":
            hi = ws.astype(ml_dtypes.bfloat16)
            lo = (ws - hi.astype(np.float32)).astype(ml_dtypes.bfloat16)
            out.append(np.ascontiguousarray(np.stack([hi, lo], axis=2)))
        else:
            out.append(np.ascontiguousarray(ws))
    return out


def pack_x(xs: np.ndarray) -> np.ndarray:
    """[BPC, 3, 640, 640] int32 -> [BPC, 3, 128, 5, 640] uint8 partition-major."""
    return np.ascontiguousarray(
        xs.reshape(BPC, CHN, 5, 128, WIMG).transpose(0, 1, 3, 2, 4).astype(np.uint8)
    )


def make_in_maps(x: np.ndarray, W: np.ndarray) -> list[dict]:
    g4, id64, iod = _make_consts()
    wps = pack_w(W)
    in_maps = []
    for c in range(NCORES):
        qmask = np.zeros((64, NQC), np.float32)
        npad = (c + 1) * NQC - NQ
        if npad > 0:
            qmask[:, NQC - min(npad, NQC):] = np.float32(NEG)
        in_maps.append(
            {
                "x": pack_x(x[c * BPC : (c + 1) * BPC]),
                "w": wps[c],
                "g4": g4,
                "id64": id64,
                "iod": iod,
                "qmask": qmask,
            }
        )
    return in_maps


def kernel(x: np.ndarray, W: np.ndarray) -> np.ndarray:
    x = np.ascontiguousarray(np.asarray(x), dtype=np.int32)
    W = np.ascontiguousarray(np.asarray(W), dtype=np.float32)
    assert x.shape == (B, CHN, HIMG, WIMG) and W.shape == (KDIM, NQ * NCHAN)

    nc = _get_nc()
    in_maps = make_in_maps(x, W)
    res = run_bass_kernel_spmd(nc, in_maps, core_ids=list(range(NCORES)))
    out = np.concatenate([res.results[c]["out"] for c in range(NCORES)], axis=0)
    return out.astype(np.float32)


if __name__ == "__main__":
    xs = np.random.randint(0, 256, (B, CHN, HIMG, WIMG)).astype(np.int32)
    Ws = (np.random.randn(KDIM, NQ * NCHAN) * 0.02).astype(np.float32)
    o = kernel(xs, Ws)
    print("kernel output:", o.shape, o.dtype)


# revision 4
# speedup vs baseline: 1.0070x; 1.0070x over previous
"""Trainium2 Bass kernel v2: end-to-end model (pool -> linear -> max/argmax ->
top-k -> gather), query-sharded across 8 NeuronCores.

Sharding: x batch-sharded (8 samples/core) for pooling; pooled features
AllGathered (307KB); W output-column (query) sharded 38-of-304/core so each
core streams only 15.3MB of W (vs 121MB replicated); per-query results
AllToAll'd (68KB) so each core runs topk+gather for its own 8 samples.

Self-contained: hardcodes all shapes; builds one SPMD Bass program and runs
it via run_bass_kernel_spmd on cores 0-7.
"""

import os
import sys
from contextlib import ExitStack

import numpy as np

for _p in ("/opt/trn_rl_repo", "/root/.axon_site/_ro/trn_rl_repo"):
    if os.path.isdir(_p) and _p not in sys.path:
        sys.path.append(_p)

import concourse.bass as bass
import concourse.tile as tile
from concourse import bacc, library_config, mybir
from concourse.bass_utils import run_bass_kernel_spmd

dt = mybir.dt
F32 = dt.float32
AX = mybir.AxisListType
OP = mybir.AluOpType

# ---------------- problem constants (hardcoded) ----------------
B, CHN, HIMG, WIMG = 64, 3, 640, 640
NQ, NCHAN, NCL, TOPK = 300, 84, 80, 150
KDIM = 1200
NCORES = 8
BPC = B // NCORES                  # samples per core = 8
KT, KTS = 10, 120                  # K tiling: 10 x 120
NQP = 304                          # padded query count (8*38)
NQC = NQP // NCORES                # queries per core = 38
NCOLS = NQC * NCHAN                # 3192 matmul columns per core
CHUNKQ = [6, 6, 6, 6, 6, 6, 2]     # queries per GEMM chunk (6*84=504 <= 512 psum)
SCALE = np.float64(1.0) / (32 * 32 * 255)
NEG = -3.0e38
NIDX = 160                         # padded top-k index count (152 used)
NROUND = 19                        # 19 rounds x 8 = 152 >= 150
FEAT = NQC * 6                     # 228 feat cols per (dest, sample)
BLK = FEAT + NQC                   # 266 f32 per all-to-all block row

# GEMM mode: "bf3" = 3-pass bf16 decomposition (y = Wh@ph + Wl@ph + Wh@pl,
# exact to ~2^-16 since bf16 products accumulate in f32 psum; 3 PE cycles/col
# vs f32's 4), "f32" = plain f32 matmul (slower, max precision)
MM_MODE = os.environ.get("KERNEL_MM", "bf3")
BF16 = dt.bfloat16


def build_program():
    nc = bacc.Bacc(
        "TRN2", target_bir_lowering=False, debug=False, num_devices=NCORES
    )
    # x host-packed to uint8 partition-major: 4x fewer HBM bytes, linear reads
    x_d = nc.dram_tensor(
        "x", [BPC, CHN, 128, 5, WIMG], dt.uint8, kind="ExternalInput"
    )
    # per-core query slice of W, K-permuted on host, tile-contiguous
    if MM_MODE == "bf3":
        w_d = nc.dram_tensor("w", [KT, KTS, 2, NCOLS], BF16, kind="ExternalInput")
    else:
        w_d = nc.dram_tensor("w", [KT, KTS, NCOLS], F32, kind="ExternalInput")
    g4_d = nc.dram_tensor("g4", [128, 4], F32, kind="ExternalInput")
    id64_d = nc.dram_tensor("id64", [64, 64], F32, kind="ExternalInput")
    iod_d = nc.dram_tensor("iod", [64, NCL], F32, kind="ExternalInput")
    qmask_d = nc.dram_tensor("qmask", [64, NQC], F32, kind="ExternalInput")
    out_d = nc.dram_tensor("out", [BPC, TOPK, 6], F32, kind="ExternalOutput")

    with tile.TileContext(nc) as tc:
        with ExitStack() as ctx:
            _body(ctx, tc, x_d, w_d, g4_d, id64_d, iod_d, qmask_d, out_d)
    nc.finalize()
    return nc


def _body(ctx, tc, x_d, w_d, g4_d, id64_d, iod_d, qmask_d, out_d):
    nc = tc.nc

    # ---------------- persistent tiles ----------------
    P = ctx.enter_context(tc.tile_pool(name="persist", bufs=1))
    DCC = ctx.enter_context(tc.tile_pool(name="dcc", bufs=1, space="DRAM"))

    g4 = P.tile([128, 4], F32, tag="g4")
    id64 = P.tile([64, 64], F32, tag="id64")
    iod = P.tile([64, NCL], F32, tag="iod")
    qmask = P.tile([64, NQC], F32, tag="qmask")

    pall_a = P.tile([32, KDIM], F32, tag="pall_a")       # pooled, images 0-3 x 8 cores
    pall_b = P.tile([32, KDIM], F32, tag="pall_b")       # pooled, images 4-7 x 8 cores
    if MM_MODE == "bf3":
        ph = [P.tile([KTS, 64], BF16, tag=f"ph{k}", name=f"ph{k}") for k in range(KT)]
        pl = [P.tile([KTS, 64], BF16, tag=f"pl{k}", name=f"pl{k}") for k in range(KT)]
        wt = [P.tile([KTS, 2 * NCOLS], BF16, tag=f"wt{k}", name=f"wt{k}") for k in range(KT)]
    else:
        pt = [P.tile([KTS, 64], F32, tag=f"pt{k}", name=f"pt{k}") for k in range(KT)]
        wt = [P.tile([KTS, NCOLS], F32, tag=f"wt{k}", name=f"wt{k}") for k in range(KT)]
    feat6 = P.tile([64, FEAT], F32, tag="feat6")         # [sample, (q, 6)] own queries
    scores6 = P.tile([64, NQC], F32, tag="scores6")      # per-query max score
    eq = P.tile([64, 6 * NCL], F32, tag="eq")            # argmax scratch
    am = P.tile([64, 6 * NCL], F32, tag="am")
    argt = P.tile([64, 6], F32, tag="argt")
    feat = P.tile([128, NQP * 6], F32, tag="feat")       # gather source [p, q, 6]
    swk = P.tile([BPC, NQP], F32, tag="swk")             # topk scratch (destroyed)
    tv = P.tile([BPC, NROUND * 8], F32, tag="tv")        # topk values (desc)
    ti = P.tile([BPC, NROUND * 8], dt.uint32, tag="ti")  # topk indices
    ti16 = P.tile([BPC, NIDX], dt.int16, tag="ti16")
    wrap = P.tile([128, NIDX // 16], dt.int16, tag="wrap")
    gout = P.tile([128, NIDX * 6], F32, tag="gout")

    # collective bounce buffers (DRAM)
    ag_in = DCC.tile([BPC, KDIM], F32, tag="ag_in")
    ag1_out = DCC.tile([32, KDIM], F32, tag="ag1_out")
    ag2_out = DCC.tile([32, KDIM], F32, tag="ag2_out")
    aas_in = DCC.tile([64, NQC], F32, tag="aas_in")      # scores-only A2A
    aas_out = DCC.tile([64, NQC], F32, tag="aas_out")
    aaf_in = DCC.tile([64, FEAT], F32, tag="aaf_in")     # feat A2A
    aaf_out = DCC.tile([64, FEAT], F32, tag="aaf_out")

    nc.vector.memset(ti16[:, :], 0)
    nc.vector.memset(feat[:, :], 0)  # only partitions 16b hold real data
    # load the gather library up front so no drain+reload lands in the tail
    nc.gpsimd.load_library(library_config.ap_gather)

    def _wdma(k, eng):
        if MM_MODE == "bf3":
            eng.dma_start(
                wt[k][:].rearrange("p (h n) -> p h n", h=2), w_d[k]
            )
        else:
            eng.dma_start(wt[k][:], w_d[k])

    # ---------------- phase 1: pooling (x -> ag_in [8,1200]) ----------------
    # K layout of a pooled row: k = i*300 + cx*100 + t*20 + j  (i=p//32,
    # cx=BGR channel, t=row tile, j=width group). The host W permutation
    # absorbs this AND the BGR->RGB flip, so the device never reorders.
    with tc.tile_pool(name="xp", bufs=12) as XP, \
         tc.tile_pool(name="s1p", bufs=6) as S1P, \
         tc.tile_pool(name="smallp", bufs=4) as SMALL, \
         tc.tile_pool(name="pps", bufs=4, space="PSUM") as PPS:
        # issue ALL x DMAs up front, alternating both HWDGE queues so the
        # first tiles land 2x sooner; nothing here may wait on pooling
        # compute, or the x stream serializes behind it
        xas = []
        for b in range(BPC):
            for cx in range(CHN):
                xa = XP.tile([128, 5 * WIMG], dt.uint8, tag="xa", name="xa")
                eng = nc.sync if (b * CHN + cx) % 2 == 0 else nc.scalar
                eng.dma_start(
                    xa[:].rearrange("p (t w) -> p t w", t=5), x_d[b, cx]
                )
                xas.append(xa)
        # constants next (needed from ~25us on; they land right behind x)
        nc.sync.dma_start(g4[:], g4_d[:])
        nc.sync.dma_start(id64[:], id64_d[:])
        nc.sync.dma_start(iod[:], iod_d[:])
        nc.sync.dma_start(qmask[:], qmask_d[:])
        # the whole W stream rides the sync queue behind x: the scalar
        # queue stays free for pooling pc/scatters (a W trigger's DGE
        # stall would delay them and so the first AllGather), and W isn't
        # consumed until after the second AllGather anyway
        for k in range(KT):
            _wdma(k, nc.sync)

        for b in range(BPC):
            ps = PPS.tile([4, 3 * 100], F32, tag="ps", name="ps")
            pc = SMALL.tile([4, 3 * 100], F32, tag="pc", name="pc")
            for cx in range(CHN):
                xa = xas[b * CHN + cx]
                s1f = S1P.tile([128, 100], F32, tag="s1f", name="s1f")
                # width pooling: sum groups of 32 -> [128, (t,j)] f32 (exact:
                # sums of 32 uint8 <= 8160 < 2^24)
                with nc.allow_low_precision(reason="f32 sums of uint8 are exact"):
                    nc.vector.tensor_reduce(
                        s1f[:],
                        xa[:].rearrange("p (t j g) -> p t j g", t=5, j=20),
                        axis=AX.X,
                        op=OP.add,
                    )
                # height pooling via PE: G4.T @ s1f -> [4, 100] (scaled)
                nc.tensor.matmul(
                    ps[:, cx * 100 : (cx + 1) * 100], g4[:], s1f[:],
                    start=True, stop=True,
                )
            nc.scalar.copy(pc[:], ps[:])
            # one scatter DMA per image, straight into the AllGather input:
            # ag_in[b] row-major = (i, cx, t, j) = the k layout above
            nc.scalar.dma_start(
                ag_in[b].rearrange("(i r) -> i r", i=4), pc[:]
            )

    # ---------------- phase 2: AllGather pooled, split in 2 ----------------
    # AG#1 covers images 0-3 and fires while images 4-7 are still pooling,
    # hiding roughly half the collective latency + cross-core skew.
    # Sample order becomes interleaved: pall_a row 4d+u = sample 8d+u,
    # pall_b row 4d+u = sample 8d+4+u; the lhsT column (= psum partition)
    # carries that order, undone by the AllToAll staging DMAs below.
    nc.gpsimd.collective_compute(
        "AllGather",
        OP.bypass,
        replica_groups=[list(range(NCORES))],
        ins=[ag_in[0:4, :]],
        outs=[ag1_out[:]],
    )
    nc.gpsimd.collective_compute(
        "AllGather",
        OP.bypass,
        replica_groups=[list(range(NCORES))],
        ins=[ag_in[4:8, :]],
        outs=[ag2_out[:]],
    )
    nc.sync.dma_start(pall_a[:], ag1_out[:])
    nc.sync.dma_start(pall_b[:], ag2_out[:])

    # transpose pall halves -> lhsT tiles [120, 64] for the main matmul
    with tc.tile_pool(name="pts", bufs=2, space="PSUM") as PTS:
        for k in range(KT):
            pst = PTS.tile([KTS, 64], F32, tag="pst", name="pst")
            nc.tensor.transpose(
                pst[:, 0:32], pall_a[:, k * KTS : (k + 1) * KTS], id64[0:32, 0:32]
            )
            nc.tensor.transpose(
                pst[:, 32:64], pall_b[:, k * KTS : (k + 1) * KTS], id64[0:32, 0:32]
            )
            # the psum->sbuf copy also un-interleaves the AG-split sample
            # order: lhsT col 8d+4*half+u <- pst col 32*half+4d+u, so the
            # GEMM's psum partitions are in true sample order downstream
            def _unperm(ap):  # [120, 64] -> [120, (half, d, u)] iteration view
                return ap.rearrange("p (d two u) -> p two d u", two=2, u=4)

            def _src(ap):
                return ap.rearrange("p (two d u) -> p two d u", d=8, u=4)

            if MM_MODE == "bf3":
                nc.scalar.copy(_unperm(ph[k][:]), _src(pst[:]))  # hi = bf16(p)
                nc.vector.tensor_tensor(                         # lo = bf16(p - hi)
                    _unperm(pl[k][:]), _src(pst[:]), _unperm(ph[k][:]),
                    op=OP.subtract,
                )
            else:
                nc.scalar.copy(_unperm(pt[k][:]), _src(pst[:]))

    # ------------- phase 3: main matmul + per-chunk postproc -----------
    fv = feat6[:].rearrange("p (q c) -> p q c", c=6)
    q0s = np.cumsum([0] + CHUNKQ).tolist()
    with tc.tile_pool(name="yps", bufs=3, space="PSUM") as YPS:
        # chunk-major: W is fully resident well before the AllGather lands,
        # so no arrival pacing is needed; chunk j's postproc (DVE) then
        # overlaps the PE running chunk j+1
        for j, nq in enumerate(CHUNKQ):
            psy = YPS.tile([64, 6 * NCHAN], F32, tag="psy", name="psy")
            c0 = q0s[j] * NCHAN
            for k in range(KT):
                if MM_MODE == "bf3":
                    for pi, (lhs, off) in enumerate(
                        [(ph[k], 0), (pl[k], 0), (ph[k], NCOLS)]
                    ):
                        nc.tensor.matmul(
                            psy[:, : nq * NCHAN],
                            lhs[:],
                            wt[k][:, off + c0 : off + c0 + nq * NCHAN],
                            start=(k == 0 and pi == 0),
                            stop=(k == KT - 1 and pi == 2),
                        )
                else:
                    nc.tensor.matmul(
                        psy[:, : nq * NCHAN],
                        pt[k][:],
                        wt[k][:, c0 : c0 + nq * NCHAN],
                        start=(k == 0),
                        stop=(k == KT - 1),
                    )
            q0 = q0s[j]
            psv = psy[:, : nq * NCHAN].rearrange("b (q c) -> b q c", q=nq)
            # boxes -> feat6[:, q, 0:4]
            nc.vector.tensor_copy(fv[:, q0 : q0 + nq, 0:4], psv[:, :, 0:4])
            # per-query max score -> scores6
            nc.vector.tensor_reduce(
                scores6[:, q0 : q0 + nq], psv[:, :, 4:NCHAN],
                axis=AX.X, op=OP.max,
            )
            # argmax id: first class attaining the max (descending-iod trick)
            eqv = eq[:, : nq * NCL].rearrange("p (q c) -> p q c", q=nq)
            nc.vector.tensor_tensor(
                eqv, psv[:, :, 4:NCHAN],
                scores6[:, q0 : q0 + nq].unsqueeze(-1).broadcast_to((64, nq, NCL)),
                op=OP.is_ge,
            )
            amv = am[:, : nq * NCL].rearrange("p (q c) -> p q c", q=nq)
            nc.vector.tensor_tensor(
                amv, eqv,
                iod[:].unsqueeze(1).broadcast_to((64, nq, NCL)),
                op=OP.mult,
            )
            nc.vector.tensor_reduce(argt[:, :nq], amv, axis=AX.X, op=OP.max)
            # id = 79 - arg -> feat6[:, q, 5]
            nc.vector.tensor_scalar(
                fv[:, q0 : q0 + nq, 5], argt[:, :nq], -1.0, float(NCL - 1),
                op0=OP.mult, op1=OP.add,
            )

    # mask pad queries (core 7 only; qmask is 0 elsewhere; adds are exact)
    nc.vector.tensor_tensor(scores6[:], scores6[:], qmask[:], op=OP.add)

    # ------------- phase 4: two AllToAlls — scores first -----------------
    # the serial topk chain only needs scores, which are final right after
    # the last chunk's reduce + qmask; boxes/ids postproc, feat staging and
    # the feat A2A all then overlap the topk rounds
    nc.sync.dma_start(aas_in[:], scores6[:])
    nc.gpsimd.collective_compute(
        "AllToAll",
        OP.bypass,
        replica_groups=[list(range(NCORES))],
        ins=[aas_in[:]],
        outs=[aas_out[:]],
    )
    nc.sync.dma_start(
        swk[:].rearrange("b (s q) -> b s q", s=NCORES),
        aas_out[:].rearrange("(s b) q -> b s q", s=NCORES),
    )

    nc.vector.tensor_copy(fv[:, :, 4], scores6[:])
    nc.scalar.dma_start(aaf_in[:], feat6[:])
    nc.gpsimd.collective_compute(
        "AllToAll",
        OP.bypass,
        replica_groups=[list(range(NCORES))],
        ins=[aaf_in[:]],
        outs=[aaf_out[:]],
    )
    # feat rows: partition 16b gets [304 queries, 6] contiguous
    nc.scalar.dma_start(
        feat[:].rearrange("(b o) f -> b o f", o=16)[:, 0, :]
               .rearrange("b (s e) -> b s e", s=NCORES),
        aaf_out[:].rearrange("(s b) e -> b s e", s=NCORES),
    )
    # reload the gather library right after the collectives: the gpsimd
    # queue drain this forces then runs during the topk rounds instead of
    # between the index wrap and ap_gather in the tail
    nc.gpsimd.load_library(library_config.ap_gather)

    # ---------------- phase 5: top-150 via iterated max8 ----------------
    # index wrap (ti16 -> DRAM -> wrap) is pipelined in 2 halves so most of
    # it hides under the later topk rounds
    with tc.tile_pool(name="dscr", bufs=1, space="DRAM") as DSCR:
        # on-chip transpose to wrap layout (one strided DVE copy), then the
        # DRAM bounce DMAs are plain 2-dim copies
        tsc = DSCR.tile([BPC, NIDX], dt.int16, tag="tsc")
        tip = P.tile([BPC, NIDX], dt.int16, tag="tip")
        for r in range(NROUND):
            nc.vector.max(tv[:, 8 * r : 8 * r + 8], swk[:, :])
            nc.vector.max_index(ti[:, 8 * r : 8 * r + 8], tv[:, 8 * r : 8 * r + 8], swk[:, :])
            if r < NROUND - 1:
                nc.vector.match_replace(
                    swk[:, :], tv[:, 8 * r : 8 * r + 8], swk[:, :], NEG
                )
        nc.vector.tensor_copy(ti16[:, : NROUND * 8], ti[:, :])
        # tip[b, p*10 + f] = ti16[b, f*16 + p]  (wrap layout, p-major)
        nc.vector.tensor_copy(
            tip[:].rearrange("b (p f) -> b p f", f=NIDX // 16),
            ti16[:].rearrange("b (f p) -> b p f", p=16),
        )
        nc.sync.dma_start(tsc[:], tip[:])
        nc.sync.dma_start(
            wrap[:], tsc[:].rearrange("b (p f) -> (b p) f", f=NIDX // 16)
        )

    # ---------------- phase 6: gather + output ----------------
    nc.gpsimd.ap_gather(
        gout[:].rearrange("p (i c) -> p i c", c=6),
        feat[:].rearrange("p (q c) -> p q c", c=6),
        wrap[:],
        channels=128,
        num_elems=NQP,
        d=6,
        num_idxs=NIDX,
    )
    # one DMA: partition 16b row -> out sample b
    nc.sync.dma_start(
        out_d[:].rearrange("b k c -> b (k c)"),
        gout[:].rearrange("(b o) f -> b o f", o=16)[:, 0, : TOPK * 6],
    )


def _k_perm():
    """k_new = i*300 + cx*100 + t*20 + j -> k_orig = (2-cx)*400 + (t*4+i)*20 + j."""
    perm = np.empty(KDIM, np.int64)
    for i in range(4):
        for cx in range(CHN):
            for t in range(5):
                for j in range(20):
                    k_new = i * 300 + cx * 100 + t * 20 + j
                    perm[k_new] = (2 - cx) * 400 + (t * 4 + i) * 20 + j
    return perm


def _make_consts():
    g4 = np.zeros((128, 4), np.float32)
    for i in range(4):
        g4[32 * i : 32 * (i + 1), i] = np.float32(SCALE)
    id64 = np.eye(64, dtype=np.float32)
    iod = np.broadcast_to(
        (np.float32(NCL - 1) - np.arange(NCL, dtype=np.float32))[None, :], (64, NCL)
    ).copy()
    return g4, id64, iod


_NC_CACHE = {}


def _get_nc():
    key = MM_MODE
    if key not in _NC_CACHE:
        _NC_CACHE[key] = build_program()
    return _NC_CACHE[key]


def pack_w(W: np.ndarray) -> list[np.ndarray]:
    """[1200, 25200] -> per-core [KT, 120, (2,) 3192], K-permuted, query-padded."""
    import ml_dtypes

    Wp = W[_k_perm()].reshape(KDIM, NQ, NCHAN)
    Wpad = np.zeros((KDIM, NQP, NCHAN), np.float32)
    Wpad[:, :NQ, :] = Wp
    out = []
    for c in range(NCORES):
        ws = Wpad[:, c * NQC : (c + 1) * NQC, :].reshape(KT, KTS, NCOLS)
        if MM_MODE == "bf3":
            hi = ws.astype(ml_dtypes.bfloat16)
            lo = (ws - hi.astype(np.float32)).astype(ml_dtypes.bfloat16)
            out.append(np.ascontiguousarray(np.stack([hi, lo], axis=2)))
        else:
            out.append(np.ascontiguousarray(ws))
    return out


def pack_x(xs: np.ndarray) -> np.ndarray:
    """[BPC, 3, 640, 640] int32 -> [BPC, 3, 128, 5, 640] uint8 partition-major."""
    return np.ascontiguousarray(
        xs.reshape(BPC, CHN, 5, 128, WIMG).transpose(0, 1, 3, 2, 4).astype(np.uint8)
    )


def make_in_maps(x: np.ndarray, W: np.ndarray) -> list[dict]:
    g4, id64, iod = _make_consts()
    wps = pack_w(W)
    in_maps = []
    for c in range(NCORES):
        qmask = np.zeros((64, NQC), np.float32)
        npad = (c + 1) * NQC - NQ
        if npad > 0:
            qmask[:, NQC - min(npad, NQC):] = np.float32(NEG)
        in_maps.append(
            {
                "x": pack_x(x[c * BPC : (c + 1) * BPC]),
                "w": wps[c],
                "g4": g4,
                "id64": id64,
                "iod": iod,
                "qmask": qmask,
            }
        )
    return in_maps


def kernel(x: np.ndarray, W: np.ndarray) -> np.ndarray:
    x = np.ascontiguousarray(np.asarray(x), dtype=np.int32)
    W = np.ascontiguousarray(np.asarray(W), dtype=np.float32)
    assert x.shape == (B, CHN, HIMG, WIMG) and W.shape == (KDIM, NQ * NCHAN)

    nc = _get_nc()
    in_maps = make_in_maps(x, W)
    res = run_bass_kernel_spmd(nc, in_maps, core_ids=list(range(NCORES)))
    out = np.concatenate([res.results[c]["out"] for c in range(NCORES)], axis=0)
    return out.astype(np.float32)


if __name__ == "__main__":
    xs = np.random.randint(0, 256, (B, CHN, HIMG, WIMG)).astype(np.int32)
    Ws = (np.random.randn(KDIM, NQ * NCHAN) * 0.02).astype(np.float32)
    o = kernel(xs, Ws)
    print("kernel output:", o.shape, o.dtype)
